# revision 21
# baseline (speedup 1.0000x reference)
"""Trainium2 Bass kernel for nn_AttnCoef (sparse attention coefficients).

Problem: alpha = softmax_masked(q @ k^T / sqrt(DH)) over Lk = n^2, with an
all-distinct index mask M(i,(j,k)) = [i!=j][i!=k][j!=k] and node-validity
masks. Output [H=4, B=4, Lq=128, Lk=16384] f32 (128 MiB).

Strategy (8 NeuronCores, data parallel over the 16 (h,b) pairs, 2 per core):
- All masking is folded into the matmul as additive -C biases so that
  exp() underflows masked entries to exactly 0:
    * lk-only mask ([j!=k] & node masks) rides an extra contraction row
      (weight 1.0, bias row -C*(1-kvalid)).
    * [i=j] block mask: 4 extra contraction rows with per-chunk one-hot
      weights (chunk = 512 lk = 4 j-blocks).
    * [i=k] strided diagonal mask: second accumulating matmul
      (-C*I as stationary, periodic identity as moving operand).
- One ScalarE pass: p = exp(0.25*psum) PSUM->SBUF (bf16) with fused
  per-row accumulation (denominators).
- recip = q_mask / (denom + eps); one VectorE tensor_scalar pass scales
  p by recip into f32; DMA out.
"""

import sys

sys.path.insert(0, "/opt/trn_rl_repo")

import numpy as np
import ml_dtypes

H, B, N, DQK, DH = 4, 4, 128, 64, 16
LK = N * N  # 16384
NCORES = 8
PAIRS_PER_CORE = 2
NCHUNK, CW = 32, 512  # matmul chunks per pair
NGRP, GW = 8, 2048  # psum groups per pair (4 chunks each)
NOUT, OW = 2, 8192  # output store chunks per pair
KDIM = DH + 1 + 4  # 21 contraction rows
BIGC = 98304.0  # additive mask constant (exact in bf16/f32)

TRACE = False
_LAST = None
_NC_CACHE = None


def _build_nc():
    import concourse.tile as tile
    from concourse import bacc, mybir

    nc = bacc.Bacc(None, target_bir_lowering=False)
    f32, bf16 = mybir.dt.float32, mybir.dt.bfloat16

    lhs_e = nc.declare_dram_parameter(
        "lhs", [PAIRS_PER_CORE, 64, NCHUNK, N], bf16, isOutput=False
    )
    rhs_e = nc.declare_dram_parameter("rhs", [N, LK], bf16, isOutput=False)
    wir_e = nc.declare_dram_parameter(
        "wir", [N, N + CW + 2 * PAIRS_PER_CORE], bf16, isOutput=False
    )
    out_e = nc.declare_dram_parameter(
        "out", [PAIRS_PER_CORE * N, LK], bf16, isOutput=True
    )

    EXP = mybir.ActivationFunctionType.Exp
    ADD = mybir.AluOpType.add
    AXX = mybir.AxisListType.X

    with tile.TileContext(nc) as tc:
        with (
            tc.tile_pool(name="consts", bufs=1) as consts,
            tc.tile_pool(name="pp", bufs=2) as pp,
            tc.tile_pool(name="psum", bufs=2, space="PSUM") as psum,
            tc.tile_pool(name="small", bufs=2) as small,
            tc.tile_pool(name="op", bufs=4) as op,
        ):
            wir_t = consts.tile([N, N + CW + 2 * PAIRS_PER_CORE], bf16)
            wid_t = wir_t[:, :N]
            irep_t = wir_t[:, N : N + CW]
            qm_t = wir_t[:, N + CW :].bitcast(f32)
            # Shared K=128 rhs tile, host-zero-padded to all 128 partitions
            # (full DMA port bandwidth): pair0 rows 0-20, pair1 rows 32-52,
            # everything else zero. Each pair's lhsT is zero outside its
            # rows, so every matmul runs with a full 128-row contraction.
            # Column-chunked DMAs so early matmul groups start sooner.
            rhs_t = consts.tile([N, LK], bf16)
            edges = [0, 2048, 4096, 8192, 12288, 16384]
            for rc in range(len(edges) - 1):
                nc.sync.dma_start(
                    out=rhs_t[:, edges[rc] : edges[rc + 1]],
                    in_=rhs_e[:][:, edges[rc] : edges[rc + 1]],
                )
            # lhs rows 64-127 are all-zero padding for every pair: memset
            # them once on-device (disjoint from the row 0-63 DMAs below,
            # so everything runs concurrently); ship rows 0-63 in chunk
            # quarters so the first matmul group unblocks early.
            lhs_t = consts.tile([N, PAIRS_PER_CORE, NCHUNK, N], bf16)
            nc.vector.memset(lhs_t[64:, :, :, :].bitcast(mybir.dt.uint32), 0)
            QC = NCHUNK // 4
            nc.scalar.dma_start(out=lhs_t[:64, 0, :QC, :], in_=lhs_e[:][0, :, :QC, :])
            nc.scalar.dma_start(out=wir_t[:], in_=wir_e[:])
            nc.scalar.dma_start(out=lhs_t[:64, 0, QC:, :], in_=lhs_e[:][0, :, QC:, :])
            nc.scalar.dma_start(out=lhs_t[:64, 1, :, :], in_=lhs_e[:][1])


            out_ap = out_e[:]

            for u in range(PAIRS_PER_CORE):
                p_t = pp.tile([N, LK], bf16, tag="p")
                dsum = small.tile([N, NGRP], f32, tag="dsum")

                for g in range(NGRP):
                    ps = psum.tile([N, GW], f32, tag="ps")
                    for cc in range(4):
                        c = 4 * g + cc
                        nc.tensor.matmul(
                            ps[:, cc * CW : (cc + 1) * CW],
                            lhs_t[:, u, c, :],
                            rhs_t[:, c * CW : (c + 1) * CW],
                            start=True,
                            stop=False,
                        )
                    for cc in range(4):
                        nc.tensor.matmul(
                            ps[:, cc * CW : (cc + 1) * CW],
                            wid_t[:],
                            irep_t[:],
                            start=False,
                            stop=True,
                        )
                    nc.scalar.activation(
                        out=p_t[:, g * GW : (g + 1) * GW],
                        in_=ps[:],
                        func=EXP,
                        scale=0.25,
                        accum_out=dsum[:, g : g + 1],
                    )

                den = small.tile([N, 1], f32, tag="den")
                nc.vector.tensor_reduce(out=den, in_=dsum[:], axis=AXX, op=ADD)
                den2 = small.tile([N, 1], f32, tag="den2")
                nc.vector.tensor_scalar_add(out=den2, in0=den, scalar1=1e-30)
                recip = small.tile([N, 1], f32, tag="recip")
                nc.vector.reciprocal(out=recip, in_=den2)
                recipf = small.tile([N, 1], f32, tag="recipf")
                nc.vector.tensor_mul(out=recipf, in0=recip, in1=qm_t[:, u : u + 1])

                # pair0 streams two big chunks on the sync ring during
                # pair1's compute; pair1 (all post-ACT, so the scalar ring
                # is hazard-free) splits finer across both rings to
                # parallelize the exposed tail.
                nout, ow = (NOUT, OW) if u == 0 else (2 * NOUT, OW // 2)
                for g in range(nout):
                    ob = op.tile([N, OW], bf16, tag="ob")
                    nc.vector.tensor_scalar_mul(
                        out=ob[:, :ow], in0=p_t[:, g * ow : (g + 1) * ow],
                        scalar1=recipf,
                    )
                    eng = nc.sync if (u == 0 or g % 2 == 0) else nc.scalar
                    eng.dma_start(
                        out=out_ap[u * N : (u + 1) * N, g * ow : (g + 1) * ow],
                        in_=ob[:, :ow],
                    )

    nc.compile()
    return nc


def _host_inputs(q_A, k_A, q_mask, k_mask):
    q_A = np.ascontiguousarray(np.asarray(q_A, dtype=np.float32))
    k_A = np.ascontiguousarray(np.asarray(k_A, dtype=np.float32))
    q_mask = np.asarray(q_mask).astype(bool)
    k_mask = np.asarray(k_mask).astype(bool)

    # [h, b, d, i] and [h, b, d, lk]
    qt = q_A.reshape(B, N, H, DH).transpose(2, 0, 3, 1)
    kt = k_A.reshape(B, LK, H, DH).transpose(2, 0, 3, 1)

    jne = ~np.eye(N, dtype=bool)
    kvalid = (k_mask & jne[None]).reshape(B, LK)  # [b, lk]
    row16 = (-BIGC) * (~kvalid).astype(np.float32)  # [b, lk]

    lk = np.arange(LK)
    # j-block bias rows (periodic in chunks of 512): -C where (lk//128)%4 == t
    jpat = np.where(
        ((lk // N) % 4)[None, :] == np.arange(4)[:, None], -BIGC, 0.0
    ).astype(np.float32)  # [4, LK]
    # per-chunk one-hot weights for the j rows: 1.0 where i == 4c+t
    ii = np.arange(N)
    cidx = np.arange(NCHUNK)
    lhsaug = (
        (ii[None, None, :] == (4 * cidx[None, :, None] + np.arange(4)[:, None, None]))
    ).astype(np.float32)  # [4, 32, 128]

    wir_core = np.concatenate(
        [
            -BIGC * np.eye(N, dtype=np.float32),
            (np.arange(CW) % N == np.arange(N)[:, None]).astype(np.float32),
        ],
        axis=1,
    ).astype(ml_dtypes.bfloat16)

    in_maps = []
    for core in range(NCORES):
        lhs_arr = np.zeros((PAIRS_PER_CORE, 64, NCHUNK, N), ml_dtypes.bfloat16)
        rhs_arr = np.zeros((N, LK), ml_dtypes.bfloat16)
        qm_arr = np.zeros((N, PAIRS_PER_CORE), np.float32)
        for u in range(PAIRS_PER_CORE):
            P = PAIRS_PER_CORE * core + u
            h, b = P // B, P % B
            r0 = u * 32  # this pair's contraction rows
            lhs_arr[u, r0 : r0 + DH, :, :] = qt[h, b][:, None, :]
            lhs_arr[u, r0 + DH, :, :] = 1.0
            lhs_arr[u, r0 + DH + 1 : r0 + KDIM, :, :] = lhsaug
            rhs_arr[r0 : r0 + DH] = kt[h, b]
            rhs_arr[r0 + DH] = row16[b]
            rhs_arr[r0 + DH + 1 : r0 + KDIM] = jpat
            qm_arr[:, u] = q_mask[b].astype(np.float32)
        wir = np.concatenate(
            [wir_core, np.ascontiguousarray(qm_arr).view(ml_dtypes.bfloat16)], axis=1
        )
        in_maps.append(
            {
                "lhs": lhs_arr,
                "rhs": rhs_arr,
                "wir": wir,
            }
        )
    return in_maps


def kernel(q_A, k_A, q_mask, k_mask):
    global _NC_CACHE, _LAST
    from concourse.bass_utils import run_bass_kernel_spmd

    if _NC_CACHE is None:
        _NC_CACHE = _build_nc()
    nc = _NC_CACHE

    in_maps = _host_inputs(q_A, k_A, q_mask, k_mask)
    res = run_bass_kernel_spmd(
        nc, in_maps, core_ids=list(range(NCORES)), trace=TRACE
    )
    _LAST = res

    alpha = np.empty((H, B, N, LK), np.float32)
    for core in range(NCORES):
        o = np.asarray(res.results[core]["out"]).astype(np.float32)
        for u in range(PAIRS_PER_CORE):
            P = PAIRS_PER_CORE * core + u
            alpha[P // B, P % B] = o[u * N : (u + 1) * N]
    return alpha
